# revision 9
# baseline (speedup 1.0000x reference)
"""Trainium2 Bass kernel for a dense transformer block.

Sequence-sharded across 8 NeuronCores: core r owns token rows
[256*r, 256*(r+1)).  One 2MB AllGather exchanges k^T and v; everything
else is row-local, so no all-reduce is needed for either output.
"""

import sys

sys.path.insert(0, "/opt/trn_rl_repo")

import numpy as np

import concourse.bass as bass
import concourse.mybir as mybir
import concourse.tile as tile
from concourse import bacc
from concourse.bass_utils import run_bass_kernel_spmd
from concourse.masks import make_identity

F32 = mybir.dt.float32
AF = mybir.ActivationFunctionType
ALU = mybir.AluOpType

N_CORES = 8
SEQ = 2048
HIDDEN = 1024
HEADS = 16
HD = 64
EXP = 4
MID = HIDDEN * EXP          # 4096
QKVP = HIDDEN * (3 + EXP)   # 7168
VP = HIDDEN * (1 + EXP)     # 5120
ROWS = SEQ // N_CORES       # 256 rows per core
EPS = 1e-5
NEG = -1e10
BOUND = 12.0                # analytic softmax shift: exp(x - bound - slope*i)


def _alibi_slopes(heads):
    def p2(n):
        start = 2.0 ** (-(2.0 ** (-(np.log2(n) - 3.0))))
        return [start * (start ** i) for i in range(n)]
    if np.log2(heads).is_integer():
        return p2(heads)
    c = 2 ** int(np.floor(np.log2(heads)))
    return p2(c) + p2(2 * c)[0::2][: heads - c]


def _bn_ln(nc, pool, x_ap, ncols, tag):
    """mean/rstd over the free dim of x_ap [128, ncols] -> (mean, rstd) [128,1]."""
    nsub = max(1, ncols // 512)
    sub = ncols // nsub
    stats = pool.tile([128, nsub, 6], F32, tag=f"st_{tag}", name=f"st_{tag}")
    xv = x_ap.rearrange("p (a b) -> p a b", a=nsub) if nsub > 1 else None
    for g in range(nsub):
        nc.vector.bn_stats(
            out=stats[:, g, :],
            in_=(xv[:, g, :] if nsub > 1 else x_ap),
        )
    mv = pool.tile([128, 2], F32, tag=f"mv_{tag}", name=f"mv_{tag}")
    nc.vector.bn_aggr(out=mv[:], in_=stats[:])
    eps = pool.tile([128, 1], F32, tag=f"eps_{tag}", name=f"eps_{tag}")
    nc.vector.memset(eps[:], EPS)
    rstd = pool.tile([128, 1], F32, tag=f"rs_{tag}", name=f"rs_{tag}")
    nc.scalar.activation(out=rstd[:], in_=mv[:, 1:2], func=AF.Sqrt, bias=eps[:])
    nc.vector.reciprocal(out=rstd[:], in_=rstd[:])
    return mv[:, 0:1], rstd[:]


def build_program():
    nc = bacc.Bacc("TRN2", target_bir_lowering=False, debug=False,
                   num_devices=N_CORES)

    # ---- per-core I/O ----
    xa = nc.dram_tensor("xa", [ROWS, HIDDEN], F32, kind="ExternalInput")
    xb = nc.dram_tensor("xb", [ROWS, HIDDEN], F32, kind="ExternalInput")
    shift_mask = nc.dram_tensor("shift_mask", [1, 1], F32, kind="ExternalInput")
    acc_exp = nc.dram_tensor("acc_exp", [ROWS, SEQ], F32, kind="ExternalInput")
    alibi = nc.dram_tensor("alibi", [HEADS, SEQ], F32, kind="ExternalInput")
    exp_bias = nc.dram_tensor("exp_bias", [2, 128, HEADS], F32, kind="ExternalInput")
    w_inT = nc.dram_tensor("w_inT", [HIDDEN, QKVP], F32, kind="ExternalInput")
    w_outT = nc.dram_tensor("w_outT", [VP + 1, HIDDEN], F32, kind="ExternalInput")
    p_in = {}
    for nm, d in [("in_g", HIDDEN), ("in_b", HIDDEN), ("q_g", HIDDEN),
                  ("q_b", HIDDEN), ("k_g", HIDDEN), ("k_b", HIDDEN),
                  ("mid_g", MID), ("mid_b", MID), ("out_g", HIDDEN),
                  ("out_b", HIDDEN)]:
        p_in[nm] = nc.dram_tensor(nm, [1, d], F32, kind="ExternalInput")
    out = nc.dram_tensor("out", [ROWS, HIDDEN], F32, kind="ExternalOutput")
    res = nc.dram_tensor("res", [ROWS, SEQ], F32, kind="ExternalOutput")

    def bcast(ap, p=128):
        return bass.AP(tensor=ap.tensor, offset=ap.offset,
                       ap=[[0, p]] + list(ap.ap[1:]))

    with tile.TileContext(nc) as tc:
        _build_tile_kernel(nc, tc, xa, xb, shift_mask, acc_exp, alibi,
                           exp_bias, w_inT, w_outT, p_in, out, res, bcast)
    nc.compile()
    return nc


def _build_tile_kernel(nc, tc, xa, xb, shift_mask, acc_exp, alibi, exp_bias,
                       w_inT, w_outT, p_in, out, res, bcast):
    from contextlib import ExitStack
    ctx = ExitStack()
    with ctx:
        # ----- pools that live the whole kernel -----
        const = ctx.enter_context(tc.tile_pool(name="const", bufs=1))
        persist = ctx.enter_context(tc.tile_pool(name="persist", bufs=1))
        dram = ctx.enter_context(tc.tile_pool(name="dram", bufs=1, space="DRAM"))

        ident = const.tile([128, 128], F32)
        make_identity(nc, ident)
        ones = const.tile([1, 256], F32)
        nc.vector.memset(ones[:], 1.0)
        ebias = const.tile([128, 2, HEADS], F32)
        nc.sync.dma_start(out=ebias[:], in_=exp_bias[:].rearrange("a p h -> p a h"))
        alibi_sb = const.tile([HEADS, SEQ], F32)
        nc.sync.dma_start(out=alibi_sb[:], in_=alibi[:])
        mask_sb = const.tile([1, 1], F32)
        nc.sync.dma_start(out=mask_sb[:], in_=shift_mask[:])
        outg_sb = const.tile([128, HIDDEN], F32, tag="outg")
        outb_sb = const.tile([128, HIDDEN], F32, tag="outb")
        nc.sync.dma_start(out=outg_sb[:], in_=bcast(p_in["out_g"][:]))
        nc.sync.dma_start(out=outb_sb[:], in_=bcast(p_in["out_b"][:]))

        # persistent SBUF tensors (span phase 1 -> 4)
        qT = [persist.tile([128, 256], F32, tag=f"qT{ct}", name=f"qT{ct}") for ct in range(8)]
        oT = [persist.tile([128, 256], F32, tag=f"oT{ct}", name=f"oT{ct}") for ct in range(8)]

        # DRAM scratch
        ag_in = dram.tile([2, HIDDEN, 256], F32)
        ag_out = dram.tile([N_CORES, 2, HIDDEN, 256], F32, addr_space="Shared")
        pT_dram = dram.tile([32, 128, 256], F32)

        # =========== Phase 1: in_ln, shift, in_proj, q/k ln, transposes ======
        with tc.tile_pool(name="ph1", bufs=1) as ph1, \
             tc.tile_pool(name="ph1w", bufs=3) as ph1w, \
             tc.tile_pool(name="wstream", bufs=6) as wstream, \
             tc.tile_pool(name="ph1ps", bufs=4, space="PSUM") as ph1ps, \
             tc.tile_pool(name="tps", bufs=2, space="PSUM") as tps:

            ing_sb = ph1.tile([128, HIDDEN], F32, tag="ing")
            inb_sb = ph1.tile([128, HIDDEN], F32, tag="inb")
            qg_sb = ph1.tile([128, HIDDEN], F32, tag="qg")
            qb_sb = ph1.tile([128, HIDDEN], F32, tag="qb")
            kg_sb = ph1.tile([128, HIDDEN], F32, tag="kg")
            kb_sb = ph1.tile([128, HIDDEN], F32, tag="kb")
            for t, nm in [(ing_sb, "in_g"), (inb_sb, "in_b"), (qg_sb, "q_g"),
                          (qb_sb, "q_b"), (kg_sb, "k_g"), (kb_sb, "k_b")]:
                nc.sync.dma_start(out=t[:], in_=bcast(p_in[nm][:]))

            # ---- in_ln on xa (full) and xb (stats full, apply first 256 cols)
            ha = []
            hb4 = []
            for it in range(2):
                t_a = ph1.tile([128, HIDDEN], F32, tag=f"xa{it}")
                nc.sync.dma_start(out=t_a[:], in_=xa[it * 128:(it + 1) * 128, :])
                m, r = _bn_ln(nc, ph1w, t_a[:], HIDDEN, f"a{it}")
                nc.vector.tensor_scalar(out=t_a[:], in0=t_a[:], scalar1=m,
                                        scalar2=r, op0=ALU.subtract, op1=ALU.mult)
                nc.vector.tensor_mul(out=t_a[:], in0=t_a[:], in1=ing_sb[:])
                nc.gpsimd.tensor_add(out=t_a[:], in0=t_a[:], in1=inb_sb[:])
                ha.append(t_a)

                t_b = ph1.tile([128, HIDDEN], F32, tag=f"xb{it}")
                nc.sync.dma_start(out=t_b[:], in_=xb[it * 128:(it + 1) * 128, :])
                m, r = _bn_ln(nc, ph1w, t_b[:], HIDDEN, f"b{it}")
                b4 = ph1.tile([128, 256], F32, tag=f"hb{it}")
                nc.vector.tensor_scalar(out=b4[:], in0=t_b[:, 0:256], scalar1=m,
                                        scalar2=r, op0=ALU.subtract, op1=ALU.mult)
                nc.vector.tensor_mul(out=b4[:], in0=b4[:], in1=ing_sb[:, 0:256])
                nc.gpsimd.tensor_add(out=b4[:], in0=b4[:], in1=inb_sb[:, 0:256])
                hb4.append(b4)
            # zero the row coming from before the sequence start (core 0 only)
            nc.vector.tensor_scalar_mul(out=hb4[0][0:1, :], in0=hb4[0][0:1, :],
                                        scalar1=mask_sb[0:1, 0:1])

            # ---- build hT [1024, 256] (channel-major)
            hT = [ph1.tile([128, 256], F32, tag=f"hT{ct}", name=f"hT{ct}") for ct in range(8)]
            for ct in range(8):
                pt = tps.tile([128, 256], F32, tag="tp1")
                for it in range(2):
                    src = hb4[it][:, ct * 128:(ct + 1) * 128] if ct < 2 \
                        else ha[it][:, ct * 128:(ct + 1) * 128]
                    nc.tensor.transpose(pt[:, it * 128:(it + 1) * 128], src, ident[:])
                nc.scalar.activation(out=hT[ct][:], in_=pt[:], func=AF.Copy)

            # ---- in_proj: qkvp[i, n] = sum_c hT[c, i] * w_inT[c, n]
            q_raw = [ph1.tile([128, HIDDEN], F32, tag=f"qr{it}", name=f"qr{it}") for it in range(2)]
            k_raw = [ph1.tile([128, HIDDEN], F32, tag=f"kr{it}", name=f"kr{it}") for it in range(2)]
            v_sb = [ph1.tile([128, HIDDEN], F32, tag=f"vr{it}", name=f"vr{it}") for it in range(2)]
            p_raw = [ph1.tile([128, MID], F32, tag=f"pr{it}", name=f"pr{it}") for it in range(2)]

            def evict_dst(nchunk, it):
                if nchunk < 2:
                    return q_raw[it][:, nchunk * 512:(nchunk + 1) * 512]
                if nchunk < 4:
                    return k_raw[it][:, (nchunk - 2) * 512:(nchunk - 1) * 512]
                if nchunk < 6:
                    return v_sb[it][:, (nchunk - 4) * 512:(nchunk - 3) * 512]
                return p_raw[it][:, (nchunk - 6) * 512:(nchunk - 5) * 512]

            # k, v first so the collective can start early; q next; p last
            for nchunk in [2, 3, 4, 5, 0, 1] + list(range(6, 14)):
                rhs_tiles = []
                for kt in range(8):
                    w = wstream.tile([128, 512], F32, tag="w_in")
                    nc.sync.dma_start(
                        out=w[:],
                        in_=w_inT[kt * 128:(kt + 1) * 128,
                                  nchunk * 512:(nchunk + 1) * 512])
                    rhs_tiles.append(w)
                for it in range(2):
                    ps = ph1ps.tile([128, 512], F32, tag="mm_in")
                    for kt in range(8):
                        nc.tensor.matmul(ps[:], hT[kt][:, it * 128:(it + 1) * 128],
                                         rhs_tiles[kt][:],
                                         start=(kt == 0), stop=(kt == 7))
                    nc.scalar.activation(out=evict_dst(nchunk, it), in_=ps[:],
                                         func=AF.Copy)

            # ---- k_ln -> kT_local -> ag_in ; v -> ag_in
            for it in range(2):
                m, r = _bn_ln(nc, ph1w, k_raw[it][:], HIDDEN, f"k{it}")
                nc.vector.tensor_scalar(out=k_raw[it][:], in0=k_raw[it][:],
                                        scalar1=m, scalar2=r,
                                        op0=ALU.subtract, op1=ALU.mult)
                nc.vector.tensor_mul(out=k_raw[it][:], in0=k_raw[it][:], in1=kg_sb[:])
                nc.gpsimd.tensor_add(out=k_raw[it][:], in0=k_raw[it][:], in1=kb_sb[:])
            for ct in range(8):
                pt = tps.tile([128, 256], F32, tag="tp1")
                for it in range(2):
                    nc.tensor.transpose(pt[:, it * 128:(it + 1) * 128],
                                        k_raw[it][:, ct * 128:(ct + 1) * 128],
                                        ident[:])
                kl = ph1.tile([128, 256], F32, tag="kTl")
                nc.scalar.activation(out=kl[:], in_=pt[:], func=AF.Copy)
                nc.sync.dma_start(out=ag_in[0, ct * 128:(ct + 1) * 128, :], in_=kl[:])
            v_view = ag_in[1].rearrange("(it p f) b -> it p (f b)", it=2, p=128)
            for it in range(2):
                nc.sync.dma_start(out=v_view[it], in_=v_sb[it][:])

            nc.gpsimd.collective_compute(
                "AllGather", ALU.bypass,
                replica_groups=[list(range(N_CORES))],
                ins=[ag_in.opt()],
                outs=[ag_out.opt()],
            )

            # ---- q_ln (scale folded into gamma/beta on host) -> qT
            for it in range(2):
                m, r = _bn_ln(nc, ph1w, q_raw[it][:], HIDDEN, f"q{it}")
                nc.vector.tensor_scalar(out=q_raw[it][:], in0=q_raw[it][:],
                                        scalar1=m, scalar2=r,
                                        op0=ALU.subtract, op1=ALU.mult)
                nc.vector.tensor_mul(out=q_raw[it][:], in0=q_raw[it][:], in1=qg_sb[:])
                nc.gpsimd.tensor_add(out=q_raw[it][:], in0=q_raw[it][:], in1=qb_sb[:])
            for ct in range(8):
                pt = tps.tile([128, 256], F32, tag="tp1")
                for it in range(2):
                    nc.tensor.transpose(pt[:, it * 128:(it + 1) * 128],
                                        q_raw[it][:, ct * 128:(ct + 1) * 128],
                                        ident[:])
                nc.scalar.activation(out=qT[ct][:], in_=pt[:], func=AF.Copy)

            # ---- mid_ln + relu on p, transpose, park in DRAM
            with tc.tile_pool(name="midb", bufs=2) as midb:
                for it in range(2):
                    m, r = _bn_ln(nc, ph1w, p_raw[it][:], MID, f"p{it}")
                    nc.vector.tensor_scalar(out=p_raw[it][:], in0=p_raw[it][:],
                                            scalar1=m, scalar2=r,
                                            op0=ALU.subtract, op1=ALU.mult)
                    for quarter in range(4):
                        gq = midb.tile([128, HIDDEN], F32, tag="mg")
                        bq = midb.tile([128, HIDDEN], F32, tag="mb")
                        sl = slice(quarter * HIDDEN, (quarter + 1) * HIDDEN)
                        nc.sync.dma_start(out=gq[:], in_=bcast(p_in["mid_g"][:, sl]))
                        nc.sync.dma_start(out=bq[:], in_=bcast(p_in["mid_b"][:, sl]))
                        nc.vector.tensor_mul(out=p_raw[it][:, sl],
                                             in0=p_raw[it][:, sl], in1=gq[:])
                        nc.vector.tensor_add(out=p_raw[it][:, sl],
                                             in0=p_raw[it][:, sl], in1=bq[:])
                    nc.scalar.activation(out=p_raw[it][:], in_=p_raw[it][:],
                                         func=AF.Relu)
                for ct in range(32):
                    pt = tps.tile([128, 256], F32, tag="tp1")
                    for it in range(2):
                        nc.tensor.transpose(pt[:, it * 128:(it + 1) * 128],
                                            p_raw[it][:, ct * 128:(ct + 1) * 128],
                                            ident[:])
                    pl = ph1.tile([128, 256], F32, tag="pTl")
                    nc.scalar.activation(out=pl[:], in_=pt[:], func=AF.Copy)
                    nc.sync.dma_start(out=pT_dram[ct], in_=pl[:])

        # =========== Phase 2a: load kT_all from the AllGather ================
        ph23 = ctx.enter_context(tc.tile_pool(name="ph23", bufs=1))
        kT_all = [ph23.tile([128, SEQ], F32, tag=f"kTa{ct}", name=f"kTa{ct}")
                  for ct in range(8)]
        acc_sb = [ph23.tile([128, SEQ], F32, tag=f"acc{it}", name=f"acc{it}")
                  for it in range(2)]
        for it in range(2):
            nc.sync.dma_start(out=acc_sb[it][:],
                              in_=acc_exp[it * 128:(it + 1) * 128, :])
        ktv = ag_out[:, 0].rearrange("r c i -> c r i")
        for ct in range(8):
            nc.sync.dma_start(out=kT_all[ct][:].rearrange("c (r i) -> c r i", r=8),
                              in_=ktv[ct * 128:(ct + 1) * 128])

        # =========== Phase 2b: logit residual = mean_h(qk) ===================
        with tc.tile_pool(name="resps", bufs=1, space="PSUM") as resps, \
             tc.tile_pool(name="ressb", bufs=2) as ressb:
            for it in range(2):
                rp = resps.tile([128, SEQ], F32, tag="rps")
                for jc in range(4):
                    for kt in range(8):
                        nc.tensor.matmul(
                            rp[:, jc * 512:(jc + 1) * 512],
                            qT[kt][:, it * 128:(it + 1) * 128],
                            kT_all[kt][:, jc * 512:(jc + 1) * 512],
                            start=(kt == 0), stop=(kt == 7))
                rs = ressb.tile([128, SEQ], F32, tag="rsb")
                nc.scalar.activation(out=rs[:], in_=rp[:], func=AF.Copy,
                                     scale=1.0 / HEADS)
                nc.sync.dma_start(out=res[it * 128:(it + 1) * 128, :], in_=rs[:])

        # =========== Phase 3: attention per head =============================
        v_flat = ag_out[:, 1].rearrange("r (p f) b -> r p f b", p=256, f=4)
        with tc.tile_pool(name="att", bufs=2) as att, \
             tc.tile_pool(name="attv", bufs=2) as attv, \
             tc.tile_pool(name="lps", bufs=1, space="PSUM") as lps, \
             tc.tile_pool(name="tps2", bufs=2, space="PSUM") as tps2, \
             tc.tile_pool(name="avps", bufs=1, space="PSUM") as avps:
            for h in range(HEADS):
                ct, half = h // 2, (h % 2) * 64
                al_row = att.tile([1, SEQ], F32, tag="al_row")
                nc.sync.dma_start(out=al_row[:], in_=alibi_sb[h:h + 1, :])
                # v [2048, 64] + ones column -> [128, 16, 65]
                vh = attv.tile([128, 16, 65], F32, tag="vh")
                vsrc = v_flat[:, :, h // 4, (h % 4) * 64:(h % 4) * 64 + 64]
                for r in range(N_CORES):
                    nc.sync.dma_start(
                        out=vh[:, r * 2:(r + 1) * 2, 0:64],
                        in_=vsrc[r].rearrange("(hf p) d -> p hf d", p=128))
                nc.vector.memset(vh[:, :, 64:65], 1.0)

                expT = att.tile([128, 16, 256], F32, tag="expT")
                for it in range(2):
                    pl = lps.tile([128, SEQ], F32, tag="pl")
                    for jc in range(4):
                        nc.tensor.matmul(
                            pl[:, jc * 512:(jc + 1) * 512],
                            qT[ct][half:half + 64, it * 128:(it + 1) * 128],
                            kT_all[ct][half:half + 64, jc * 512:(jc + 1) * 512],
                            start=True, stop=False)
                        nc.tensor.matmul(
                            pl[:, jc * 512:(jc + 1) * 512],
                            ones[0:1, it * 128:(it + 1) * 128],
                            al_row[0:1, jc * 512:(jc + 1) * 512],
                            start=False, stop=False)
                        nc.tensor.matmul(
                            pl[:, jc * 512:(jc + 1) * 512],
                            ident[:],
                            acc_sb[it][:, jc * 512:(jc + 1) * 512],
                            start=False, stop=True)
                    ev = att.tile([128, SEQ], F32, tag="ev")
                    nc.scalar.activation(out=ev[:], in_=pl[:], func=AF.Exp,
                                         bias=ebias[:, it, h:h + 1])
                    for g in range(4):
                        pt = tps2.tile([128, 512], F32, tag="tp2")
                        for b in range(4):
                            nc.tensor.transpose(
                                pt[:, b * 128:(b + 1) * 128],
                                ev[:, (g * 4 + b) * 128:(g * 4 + b + 1) * 128],
                                ident[:])
                        eng = nc.vector if (g % 2 == 0) else nc.scalar
                        dst = expT[:, g * 4:(g + 1) * 4, it * 128:(it + 1) * 128]
                        if g % 2 == 0:
                            nc.vector.tensor_copy(
                                out=dst, in_=pt[:].rearrange("p (a b) -> p a b", a=4))
                        else:
                            nc.scalar.activation(
                                out=dst, in_=pt[:].rearrange("p (a b) -> p a b", a=4),
                                func=AF.Copy)
                ov = avps.tile([65, 256], F32, tag="ov")
                for jt in range(16):
                    nc.tensor.matmul(ov[:], vh[:, jt, :], expT[:, jt, :],
                                     start=(jt == 0), stop=(jt == 15))
                rcp = att.tile([1, 256], F32, tag="rcp")
                nc.vector.reciprocal(out=rcp[:], in_=ov[64:65, :])
                rb = avps.tile([64, 256], F32, tag="rb")
                nc.tensor.matmul(rb[:], ones[0:1, 0:64], rcp[0:1, :],
                                 start=True, stop=True)
                nc.scalar.activation(out=oT[ct][half:half + 64, :],
                                     in_=ov[0:64, :], func=AF.Copy)
                nc.vector.tensor_mul(out=oT[ct][half:half + 64, :],
                                     in0=oT[ct][half:half + 64, :], in1=rb[:])

        # =========== Phase 4: out_proj + out_ln ==============================
        with tc.tile_pool(name="wstr2", bufs=6) as wstr2, \
             tc.tile_pool(name="pstr", bufs=4) as pstr, \
             tc.tile_pool(name="ops", bufs=2, space="PSUM") as ops, \
             tc.tile_pool(name="oln", bufs=2) as oln:
            for it in range(2):
                po = ops.tile([128, HIDDEN], F32, tag="po")
                for ncol in range(2):
                    csl = slice(ncol * 512, (ncol + 1) * 512)
                    for kt in range(41):
                        if kt < 40:
                            w = wstr2.tile([128, 512], F32, tag="w_out")
                            nc.sync.dma_start(
                                out=w[:],
                                in_=w_outT[kt * 128:(kt + 1) * 128, csl])
                            if kt < 8:
                                lhs = oT[kt][:, it * 128:(it + 1) * 128]
                            else:
                                pt = pstr.tile([128, 256], F32, tag="pTr")
                                nc.sync.dma_start(out=pt[:], in_=pT_dram[kt - 8])
                                lhs = pt[:, it * 128:(it + 1) * 128]
                            nc.tensor.matmul(po[:, csl], lhs, w[:],
                                             start=(kt == 0), stop=False)
                        else:
                            wb = wstr2.tile([1, 512], F32, tag="w_bias")
                            nc.sync.dma_start(out=wb[:], in_=w_outT[VP:VP + 1, csl])
                            nc.tensor.matmul(po[:, csl],
                                             ones[0:1, it * 128:(it + 1) * 128],
                                             wb[:], start=False, stop=True)
                m, r = _bn_ln(nc, oln, po[:], HIDDEN, f"o{it}")
                ot = oln.tile([128, HIDDEN], F32, tag="otile")
                nc.vector.tensor_scalar(out=ot[:], in0=po[:], scalar1=m,
                                        scalar2=r, op0=ALU.subtract, op1=ALU.mult)
                nc.vector.tensor_mul(out=ot[:], in0=ot[:], in1=outg_sb[:])
                nc.vector.tensor_add(out=ot[:], in0=ot[:], in1=outb_sb[:])
                nc.sync.dma_start(out=out[it * 128:(it + 1) * 128, :], in_=ot[:])


_CACHED = {}


def _get_program():
    if "nc" not in _CACHED:
        _CACHED["nc"] = build_program()
    return _CACHED["nc"]


def _prep_inputs(x, accumulated_logits, in_proj_w, out_proj_w, out_proj_b,
                 in_ln_g, in_ln_b, q_ln_g, q_ln_b, k_ln_g, k_ln_b,
                 mid_ln_g, mid_ln_b, out_ln_g, out_ln_b):
    f = np.float32
    x2 = np.asarray(x, f).reshape(SEQ, HIDDEN)
    acc = np.asarray(accumulated_logits, f).reshape(SEQ, SEQ)
    idx = np.arange(SEQ)
    causal = np.where(idx[:, None] >= idx[None, :], 0.0, NEG).astype(f)
    acc_plus_full = acc + causal
    slopes = np.asarray(_alibi_slopes(HEADS), f)
    alibi_np = (idx[None, :].astype(f) * slopes[:, None]).astype(f)
    scale = np.float32(HD ** -0.5)
    w_inT = np.ascontiguousarray(np.asarray(in_proj_w, f).T)
    w_outT = np.ascontiguousarray(
        np.concatenate([np.asarray(out_proj_w, f).T,
                        np.asarray(out_proj_b, f)[None, :]], axis=0))
    shared = {
        "alibi": alibi_np,
        "w_inT": w_inT,
        "w_outT": w_outT,
        "in_g": np.asarray(in_ln_g, f)[None, :],
        "in_b": np.asarray(in_ln_b, f)[None, :],
        "q_g": (np.asarray(q_ln_g, f) * scale)[None, :],
        "q_b": (np.asarray(q_ln_b, f) * scale)[None, :],
        "k_g": np.asarray(k_ln_g, f)[None, :],
        "k_b": np.asarray(k_ln_b, f)[None, :],
        "mid_g": np.asarray(mid_ln_g, f)[None, :],
        "mid_b": np.asarray(mid_ln_b, f)[None, :],
        "out_g": np.asarray(out_ln_g, f)[None, :],
        "out_b": np.asarray(out_ln_b, f)[None, :],
    }
    in_maps = []
    for r in range(N_CORES):
        s = r * ROWS
        xb = np.zeros((ROWS, HIDDEN), f)
        if r == 0:
            xb[1:] = x2[0:ROWS - 1]
        else:
            xb[:] = x2[s - 1:s + ROWS - 1]
        i_glob = (s + np.arange(ROWS)).astype(f)
        eb = -(BOUND + slopes[None, :] * i_glob[:, None])      # [256, 16]
        m = dict(shared)
        m.update({
            "xa": np.ascontiguousarray(x2[s:s + ROWS]),
            "xb": xb,
            "shift_mask": np.array([[0.0 if r == 0 else 1.0]], f),
            "acc_exp": np.ascontiguousarray(acc_plus_full[s:s + ROWS]),
            "exp_bias": np.ascontiguousarray(eb.reshape(2, 128, HEADS)),
        })
        in_maps.append(m)
    return in_maps


def kernel(**inputs):
    nc = _get_program()
    in_maps = _prep_inputs(**inputs)
    result = run_bass_kernel_spmd(nc, in_maps, core_ids=list(range(N_CORES)))
    outs = [result.results[r]["out"] for r in range(N_CORES)]
    ress = [result.results[r]["res"] for r in range(N_CORES)]
    out_full = np.concatenate(outs, axis=0).reshape(1, SEQ, HIDDEN)
    res_full = np.concatenate(ress, axis=0).reshape(1, 1, SEQ, SEQ)
    return out_full, res_full
